# revision 32
# baseline (speedup 1.0000x reference)
"""Causal self-attention on 8 axon-tunneled TRN2 NeuronCores.

Sharding: core c -> (batch b = c//2, head-group g = c%2); host sums the two
head-group partial yT outputs per batch. bf16 storage / fp32 PSUM accumulate
(bf16 matmuls stream 1 row/cycle like f32r, but halve DMA, SBUF and DVE
cost). Transpose-free S^T attention layout; softmax denominator via a
ones-column in v. Per (head-pair, 512-i-window) attention units with
causally-trimmed diagonal tiles; two heads packed per score PSUM tile so one
exp covers both; additive -1e9 causal mask applied in PSUM (Pool engine) at
unit START (diagonal tiles are emitted first so mask+exp latency hides under
full-block score streaming; their AVs are deferred to the unit end). Softmax
normalize is lazy and fully off-PE: the unnormalized PSUM tile is drained to
SBUF immediately (releasing the bank for the next unit), then DVE reciprocal
-> gpsimd partition_broadcast -> DVE multiply off the critical path.
Out-proj tiles are software-pipelined into the next attention window's unit
stream; the last window's pair-3 B-half feeds out-proj via a K=64 matmul
from the staging tile (skipping the cross-partition DMA on the tail)."""
import numpy as np

B, T, D = 4, 2048, 1024
NH_LOCAL = 8
HD = 64
CL = 512
P = 128
CC = D // P
NPAIR = 4
NW = 4
W = 512

_CACHE = {}


def _emit_qkv(nc, tc, mybir, r, xT_sb, wq_sb, wk_sb, wv_sb, qT_sb, kT_sb, v_sb):
    f32 = mybir.dt.float32
    COPY = mybir.ActivationFunctionType.Copy
    with tc.tile_pool(name=f"p1ps{r}", bufs=6, space="PSUM") as pps:
        for w in range(NW):
            t0 = w * W
            for w_sb, dst, tag in ((wq_sb, qT_sb, "q"), (wk_sb, kT_sb, "k")):
                for p in range(NPAIR):
                    ps = pps.tile([P, W], f32, tag="pq")
                    for cc in range(CC):
                        nc.tensor.matmul(
                            ps[:], w_sb[:, cc, p * P:(p + 1) * P],
                            xT_sb[:, cc, t0:t0 + W],
                            start=(cc == 0), stop=(cc == CC - 1))
                    if tag == "q":
                        nc.vector.tensor_copy(dst[:, p, t0:t0 + W], ps[:])
                    else:
                        nc.scalar.activation(dst[:, p, t0:t0 + W], ps[:], COPY)
            for tcb in range(4):
                t_c = w * 4 + tcb
                ps = pps.tile([P, W], f32, tag="pq")
                for cc in range(CC):
                    nc.tensor.matmul(
                        ps[:], xT_sb[:, cc, t_c * P:(t_c + 1) * P],
                        wv_sb[:, cc, :],
                        start=(cc == 0), stop=(cc == CC - 1))
                nc.vector.tensor_copy(
                    v_sb[:, t_c, :, 0:HD],
                    ps[:].rearrange("p (h d) -> p h d", h=NH_LOCAL))


def _emit_attention(nc, tc, mybir, r, qT_sb, kT_sb, v_sb, ml_sb, mr_sb,
                    ones_sb, aT_sb, wo_sb, woB_sb, yT_r):
    f32 = mybir.dt.float32
    bf16 = mybir.dt.bfloat16
    EXP = mybir.ActivationFunctionType.Exp
    MULT = mybir.AluOpType.mult
    with tc.tile_pool(name=f"p2st{r}", bufs=2, space="PSUM") as pst, \
         tc.tile_pool(name=f"p2aT{r}", bufs=1, space="PSUM") as paT, \
         tc.tile_pool(name=f"p2py{r}", bufs=2, space="PSUM") as ppy, \
         tc.tile_pool(name=f"p2pt{r}", bufs=4) as ppt, \
         tc.tile_pool(name=f"p2n{r}", bufs=2) as pn, \
         tc.tile_pool(name=f"p2y{r}", bufs=4) as pys:
        op_queue = []
        t64_last = [None]

        def emit_op(n):
            for _ in range(n):
                if not op_queue:
                    return
                fc, t0, last_w = op_queue.pop(0)
                py = ppy.tile([P, W], f32, tag="py")
                for pair in range(NPAIR - 1):
                    nc.tensor.matmul(
                        py[:], wo_sb[:, pair, fc * P:(fc + 1) * P],
                        aT_sb[:, pair, t0:t0 + W],
                        start=(pair == 0), stop=False)
                if last_w:
                    # pair 3: A half from aT_sb, B half straight from the
                    # staging tile (skips the cross-partition DMA)
                    nc.tensor.matmul(
                        py[:], wo_sb[0:HD, 3, fc * P:(fc + 1) * P],
                        aT_sb[0:HD, 3, t0:t0 + W], start=False, stop=False)
                    nc.tensor.matmul(
                        py[:], woB_sb[:, 3, fc * P:(fc + 1) * P],
                        t64_last[0][:], start=False, stop=True)
                else:
                    nc.tensor.matmul(
                        py[:], wo_sb[:, NPAIR - 1, fc * P:(fc + 1) * P],
                        aT_sb[:, NPAIR - 1, t0:t0 + W],
                        start=False, stop=True)
                yst = pys.tile([P, W], bf16, tag="yst")
                nc.vector.tensor_copy(yst[:], py[:])
                nc.sync.dma_start(yT_r[:, fc, t0:t0 + W], yst[:])

        for w in range(NW):
            t0 = w * W
            nfull = 4 * w
            for p in range(NPAIR):
                aTA = paT.tile([HD + 1, W], f32, tag="aTA",
                               name=f"aTA_{p}_{w}")
                aTB = paT.tile([HD + 1, W], f32, tag="aTB",
                               name=f"aTB_{p}_{w}")

                def av(ptj, jc, c0, wd):
                    for par, aT in ((0, aTA), (1, aTB)):
                        p0 = W - wd if par == 0 else W
                        nc.tensor.matmul(
                            aT[:, c0:W], v_sb[:, jc, 2 * p + par, :],
                            ptj[:, p0:p0 + wd],
                            start=(jc == 0), stop=(jc == nfull + 3))

                # full blocks then diagonal tiles, one-deep AV pipeline:
                # AV(k) is emitted after exp(k+1) so the PE streams
                # scores(k+1) while ACT computes exp(k). The causal mask is
                # a rank-127 matmul (-1e9*max(0, j-i) = ml^T @ mr)
                # accumulated into the diagonal PSUM block — no cross-engine
                # hop before the exp.
                pending = None
                for jc in range(nfull):
                    stt = pst.tile([P, 2 * W], f32, tag="st",
                                   name=f"stf{jc}_{p}_{w}")
                    for par, prow in ((0, 0), (1, HD)):
                        nc.tensor.matmul(
                            stt[:, par * W:(par + 1) * W],
                            kT_sb[prow:prow + HD, p, jc * P:(jc + 1) * P],
                            qT_sb[prow:prow + HD, p, t0:t0 + W],
                            start=True, stop=True)
                    pt = ppt.tile([P, 2 * W], bf16, tag="pt")
                    nc.scalar.activation(pt[:], stt[:], EXP, scale=0.125)
                    if pending is not None:
                        av(*pending)
                    pending = (pt, jc, 0, W)
                for d in range(4):
                    jc = nfull + d
                    wd = W - d * P
                    # a matmul output may not cross a PSUM bank boundary:
                    # right-align head A against col 512, put head B after
                    # it — contiguous [512-wd, 512+wd) for a single exp.
                    stt = pst.tile([P, 2 * W], f32, tag="st",
                                   name=f"std{d}_{p}_{w}")
                    for par, prow in ((0, 0), (1, HD)):
                        c0p = W - wd if par == 0 else W
                        nc.tensor.matmul(
                            stt[:, c0p:c0p + wd],
                            kT_sb[prow:prow + HD, p, jc * P:(jc + 1) * P],
                            qT_sb[prow:prow + HD, p, t0 + d * P:t0 + W],
                            start=True, stop=False)
                        nc.tensor.matmul(
                            stt[:, c0p:c0p + P],
                            ml_sb[:], mr_sb[:], start=False, stop=True)
                    pt = ppt.tile([P, 2 * W], bf16, tag="pt")
                    nc.scalar.activation(pt[:, W - wd:W + wd],
                                         stt[:, W - wd:W + wd],
                                         EXP, scale=0.125)
                    if pending is not None:
                        av(*pending)
                    pending = (pt, jc, d * P, wd)
                av(*pending)

                # lazy normalize: drain PSUM to SBUF right away (frees the
                # bank for the next unit), then 1/s entirely off-PE.
                aTuA = pn.tile([HD + 1, W], bf16, tag="aTuA")
                nc.vector.tensor_copy(aTuA[:], aTA[:])
                aTuB = pn.tile([HD + 1, W], bf16, tag="aTuB")
                nc.vector.tensor_copy(aTuB[:], aTB[:])
                rrA = pn.tile([P, W], bf16, tag="rrA")
                rrB = pn.tile([P, W], bf16, tag="rrB")
                with nc.allow_low_precision(reason="bf16 softmax denom"):
                    nc.vector.reciprocal(rrA[HD:HD + 1, :],
                                         aTuA[HD:HD + 1, :])
                    nc.vector.reciprocal(rrB[HD:HD + 1, :],
                                         aTuB[HD:HD + 1, :])
                # 1/s broadcast to 64 partitions via K=1 PE matmul (out in
                # PSUM, multiplied from there by the DVE directly)
                rbA = ppy.tile([P, W], f32, tag="py")
                nc.tensor.matmul(rbA[0:HD, :], ones_sb[HD:HD + 1, :],
                                 rrA[HD:HD + 1, :], start=True, stop=True)
                rbB = ppy.tile([P, W], f32, tag="py")
                nc.tensor.matmul(rbB[0:HD, :], ones_sb[HD:HD + 1, :],
                                 rrB[HD:HD + 1, :], start=True, stop=True)
                nc.vector.tensor_tensor(
                    aT_sb[0:HD, p, t0:t0 + W], aTuA[0:HD, :],
                    rbA[0:HD, :], MULT)
                t64 = pn.tile([HD, W], bf16, tag="t64")
                nc.vector.tensor_tensor(t64[:], aTuB[0:HD, :],
                                        rbB[0:HD, :], MULT)
                if w == NW - 1 and p == NPAIR - 1:
                    t64_last[0] = t64
                else:
                    nc.sync.dma_start(aT_sb[HD:P, p, t0:t0 + W], t64[:])

                if w > 0:
                    emit_op(2)
            op_queue += [(fc, t0, w == NW - 1) for fc in range(CC)]
        emit_op(len(op_queue))


def _build(repeats=1):
    import concourse.bacc as bacc
    import concourse.mybir as mybir
    import concourse.tile as tile
    from contextlib import ExitStack

    f32 = mybir.dt.float32
    bf16 = mybir.dt.bfloat16

    nc = bacc.Bacc("TRN2", target_bir_lowering=False, debug=False)

    xT = nc.dram_tensor("xT", (D, T), bf16, kind="ExternalInput")
    wqT = nc.dram_tensor("wqT", (D, CL), bf16, kind="ExternalInput")
    wkT = nc.dram_tensor("wkT", (D, CL), bf16, kind="ExternalInput")
    wvT = nc.dram_tensor("wvT", (D, CL), bf16, kind="ExternalInput")
    woT = nc.dram_tensor("woT", (CL, D), bf16, kind="ExternalInput")
    ml = nc.dram_tensor("ml", (P, P), bf16, kind="ExternalInput")
    mr = nc.dram_tensor("mr", (P, P), bf16, kind="ExternalInput")
    yT = nc.dram_tensor("yT", (D, T), bf16, kind="ExternalOutput")

    xT_r = xT.ap().rearrange("(o p) t -> p o t", p=P)
    wqT_r = wqT.ap().rearrange("(o p) f -> p o f", p=P)
    wkT_r = wkT.ap().rearrange("(o p) f -> p o f", p=P)
    wvT_r = wvT.ap().rearrange("(o p) f -> p o f", p=P)
    woT_r = woT.ap().rearrange("(o p) f -> p o f", p=P)
    yT_r = yT.ap().rearrange("(o p) t -> p o t", p=P)

    with tile.TileContext(nc) as tc, ExitStack() as outer:
        persist = outer.enter_context(tc.tile_pool(name="persist", bufs=1))
        xT_sb = persist.tile([P, CC, T], bf16, tag="xT")
        qT_sb = persist.tile([P, NPAIR, T], bf16, tag="qT")
        kT_sb = persist.tile([P, NPAIR, T], bf16, tag="kT")
        aT_sb = persist.tile([P, NPAIR, T], bf16, tag="aT")
        v_sb = persist.tile([P, 16, NH_LOCAL, HD + 1], bf16, tag="v")
        wq_sb = persist.tile([P, CC, CL], bf16, tag="wq")
        wk_sb = persist.tile([P, CC, CL], bf16, tag="wk")
        wv_sb = persist.tile([P, CC, CL], bf16, tag="wv")
        wo_sb = persist.tile([P, NPAIR, D], bf16, tag="wo")
        woB_sb = persist.tile([HD, NPAIR, D], bf16, tag="woB")
        ml_sb = persist.tile([P, P], bf16, tag="ml")
        mr_sb = persist.tile([P, P], bf16, tag="mr")
        ones_sb = persist.tile([P, HD], bf16, tag="ones")
        nc.vector.memset(ones_sb[:], 1.0)

        for r in range(repeats):
            nc.gpsimd.dma_start(xT_sb[:, :, 0:W], xT_r[:, :, 0:W])
            if r == 0:
                # weights + constants load once and stay resident
                nc.sync.dma_start(wq_sb[:], wqT_r)
                nc.sync.dma_start(wk_sb[:], wkT_r)
                nc.scalar.dma_start(wv_sb[:], wvT_r)
                nc.scalar.dma_start(ml_sb[:], ml.ap())
                nc.scalar.dma_start(mr_sb[:], mr.ap())
                nc.scalar.dma_start(wo_sb[:], woT_r)
                nc.scalar.dma_start(woB_sb[:], woT_r[HD:P, :, :])
            for w in range(1, NW):
                nc.gpsimd.dma_start(xT_sb[:, :, w * W:(w + 1) * W],
                                    xT_r[:, :, w * W:(w + 1) * W])
            nc.vector.memset(v_sb[:, :, :, HD:HD + 1], 1.0)
            _emit_qkv(nc, tc, mybir, r, xT_sb, wq_sb, wk_sb, wv_sb,
                      qT_sb, kT_sb, v_sb)
            _emit_attention(nc, tc, mybir, r, qT_sb, kT_sb, v_sb, ml_sb,
                            mr_sb, ones_sb, aT_sb, wo_sb, woB_sb, yT_r)

    nc.compile()
    return nc


def _host_inputs(x, w_qkv, w_out, core):
    import ml_dtypes

    bf = ml_dtypes.bfloat16
    b, g = core // 2, core % 2
    sl = slice(CL * g, CL * g + CL)
    k_i = np.arange(P)
    # exact boolean causal mask as a rank-127 product:
    #   ml[k,j] = -400*[j == k+1],  mr[k,i] = [i <= k]
    #   => (ml^T @ mr)[j,i] = -400*[j > i]
    # -400 is dead after the 0.125 exp scale (e^-45) yet keeps the ACT exp
    # table input bounded (a -3e3*(j-i) ramp reaches -4.7e4 and breaks the
    # HW exp, which CoreSim's np.exp does not catch).
    ml = np.where(k_i[None, :] == k_i[:, None] + 1, -400.0, 0.0)
    mr = np.where(k_i[None, :] <= k_i[:, None], 1.0, 0.0)
    return {
        "xT": np.ascontiguousarray(x[b].T).astype(bf),
        "wqT": np.ascontiguousarray(w_qkv[0 * D:1 * D][sl].T).astype(bf),
        "wkT": np.ascontiguousarray(w_qkv[1 * D:2 * D][sl].T).astype(bf),
        "wvT": np.ascontiguousarray(w_qkv[2 * D:3 * D][sl].T).astype(bf),
        "woT": np.ascontiguousarray(w_out[:, sl].T).astype(bf),
        "ml": ml.astype(bf),
        "mr": mr.astype(bf),
    }


def kernel(x, w_qkv, w_out):
    from concourse import bass_utils

    if "nc" not in _CACHE:
        _CACHE["nc"] = _build()
    nc = _CACHE["nc"]

    x = np.asarray(x, dtype=np.float32)
    w_qkv = np.asarray(w_qkv, dtype=np.float32)
    w_out = np.asarray(w_out, dtype=np.float32)

    in_maps = [_host_inputs(x, w_qkv, w_out, c) for c in range(8)]
    res = bass_utils.run_bass_kernel_spmd(nc, in_maps, core_ids=list(range(8)))
    outs = res.results

    y = np.empty((B, T, D), dtype=np.float32)
    for b in range(B):
        y[b] = (outs[2 * b]["yT"].astype(np.float32)
                + outs[2 * b + 1]["yT"].astype(np.float32)).T
    return y


# revision 49
# speedup vs baseline: 1.6454x; 1.6454x over previous
"""Causal self-attention on 8 axon-tunneled TRN2 NeuronCores.

Sharding: core c -> (batch b = c//2, head-group g = c%2); host sums the two
head-group partial yT outputs per batch. bf16 storage / fp32 PSUM accumulate
(bf16 matmuls stream 1 row/cycle like f32r, but halve DMA, SBUF and DVE
cost). Transpose-free S^T attention layout; softmax denominator via a
ones-column in v.

Attention runs per (head-pair, 512-i-window) unit with causally-trimmed
diagonal tiles; two heads pack into one score PSUM tile (head A right-aligned
against the bank boundary — matmul outputs may not cross a PSUM bank) so one
exp covers both. The causal mask is an exact rank-127 PE matmul
(ml^T@mr = -400*[j>i]) accumulated into the diagonal PSUM block: no
cross-engine hop before the exp, and the exp input stays in the HW exp
table's sane range. Softmax normalize is lazy and fully off-PE: the
unnormalized PSUM tile drains to SBUF immediately (freeing the bank for the
next unit), then DVE reciprocal -> K=1 PE broadcast matmul -> DVE multiply
run off the critical path (gpsimd partition_broadcast yields garbage on HW).

Window-0 attention units are interleaved into the (PE-bound) QKV phase so
their exp/normalize hide under QKV streaming; out-proj tiles are
software-pipelined into the next window's unit stream; the last window's
pair-3 B-half feeds out-proj via a K=64 matmul from the staging tile
(skipping the cross-partition DMA on the tail). Weights/constants are loaded
once and stay resident across repeats."""
import numpy as np

B, T, D = 4, 2048, 1024
NH_LOCAL = 8
HD = 64
CL = 512
P = 128
CC = D // P
NPAIR = 4
NW = 4
W = 512

_CACHE = {}


def _emit_body(nc, tc, mybir, r, sb):
    f32 = mybir.dt.float32
    bf16 = mybir.dt.bfloat16
    COPY = mybir.ActivationFunctionType.Copy
    EXP = mybir.ActivationFunctionType.Exp
    MULT = mybir.AluOpType.mult
    (xT_sb, qT_sb, kT_sb, aT_sb, v_sb, wq_sb, wk_sb, wv_sb, wo_sb, woB_sb,
     ml_sb, mr_sb, ones_sb, yT_r) = sb

    from contextlib import ExitStack
    with ExitStack() as ctx:
        pst = ctx.enter_context(
            tc.tile_pool(name=f"pst{r}", bufs=2, space="PSUM"))
        paT = ctx.enter_context(
            tc.tile_pool(name=f"paT{r}", bufs=1, space="PSUM"))
        ppt = ctx.enter_context(tc.tile_pool(name=f"ppt{r}", bufs=4))
        pn = ctx.enter_context(tc.tile_pool(name=f"pn{r}", bufs=2))
        pys = ctx.enter_context(tc.tile_pool(name=f"pys{r}", bufs=4))

        t64_last = [None]

        def emit_unit(p, w, prb, rbtag, prev_tail=None):
            t0 = w * W
            nfull = 4 * w
            aTA = paT.tile([HD + 1, W], f32, tag="aTA", name=f"aTA_{p}_{w}")
            aTB = paT.tile([HD + 1, W], f32, tag="aTB", name=f"aTB_{p}_{w}")

            def av(ptj, jc, c0, wd):
                for par, aT in ((0, aTA), (1, aTB)):
                    p0 = W - wd if par == 0 else W
                    nc.tensor.matmul(
                        aT[:, c0:W], v_sb[:, jc, 2 * p + par, :],
                        ptj[:, p0:p0 + wd],
                        start=(jc == 0), stop=(jc == nfull + 3))

            # full blocks then diagonal tiles, one-deep AV pipeline: AV(k)
            # is emitted after exp(k+1) so the PE streams scores(k+1) while
            # ACT computes exp(k)
            pending = None
            for jc in range(nfull):
                stt = pst.tile([P, 2 * W], f32, tag="st",
                               name=f"stf{jc}_{p}_{w}")
                for par, prow in ((0, 0), (1, HD)):
                    nc.tensor.matmul(
                        stt[:, par * W:(par + 1) * W],
                        kT_sb[prow:prow + HD, p, jc * P:(jc + 1) * P],
                        qT_sb[prow:prow + HD, p, t0:t0 + W],
                        start=True, stop=True)
                pt = ppt.tile([P, 2 * W], bf16, tag="pt")
                nc.scalar.activation(pt[:], stt[:], EXP, scale=0.125)
                if jc == 0 and prev_tail is not None:
                    prev_tail()
                if pending is not None:
                    av(*pending)
                pending = (pt, jc, 0, W)
            for d in range(4):
                jc = nfull + d
                wd = W - d * P
                stt = pst.tile([P, 2 * W], f32, tag="st",
                               name=f"std{d}_{p}_{w}")
                for par, prow in ((0, 0), (1, HD)):
                    c0p = W - wd if par == 0 else W
                    nc.tensor.matmul(
                        stt[:, c0p:c0p + wd],
                        kT_sb[prow:prow + HD, p, jc * P:(jc + 1) * P],
                        qT_sb[prow:prow + HD, p, t0 + d * P:t0 + W],
                        start=True, stop=False)
                    nc.tensor.matmul(
                        stt[:, c0p:c0p + P],
                        ml_sb[:], mr_sb[:], start=False, stop=True)
                pt = ppt.tile([P, 2 * W], bf16, tag="pt")
                nc.scalar.activation(pt[:, W - wd:W + wd],
                                     stt[:, W - wd:W + wd],
                                     EXP, scale=0.125)
                if nfull == 0 and d == 0 and prev_tail is not None:
                    prev_tail()
                if pending is not None:
                    av(*pending)
                pending = (pt, jc, d * P, wd)
            av(*pending)

            # lazy normalize: drain PSUM to SBUF right away (frees the bank
            # for the next unit), then 1/s entirely off-PE
            aTuA = pn.tile([HD + 1, W], bf16, tag="aTuA")
            nc.vector.tensor_copy(aTuA[:], aTA[:])
            aTuB = pn.tile([HD + 1, W], bf16, tag="aTuB")
            nc.vector.tensor_copy(aTuB[:], aTB[:])
            rrA = pn.tile([P, W], bf16, tag="rrA")
            rrB = pn.tile([P, W], bf16, tag="rrB")
            with nc.allow_low_precision(reason="bf16 softmax denom"):
                nc.vector.reciprocal(rrA[HD:HD + 1, :], aTuA[HD:HD + 1, :])
                nc.vector.reciprocal(rrB[HD:HD + 1, :], aTuB[HD:HD + 1, :])

            # deferred normalize tail: 1/s broadcast via K=1 PE matmul
            # (gpsimd partition_broadcast produces garbage on HW), DVE
            # multiply straight from PSUM. Emitted inside the NEXT unit's
            # stream so the in-order PE never waits on the recip chain.
            def tail():
                rbA = prb.tile([P, W], f32, tag=rbtag)
                nc.tensor.matmul(rbA[0:HD, :], ones_sb[HD:HD + 1, :],
                                 rrA[HD:HD + 1, :], start=True, stop=True)
                rbB = prb.tile([P, W], f32, tag=rbtag)
                nc.tensor.matmul(rbB[0:HD, :], ones_sb[HD:HD + 1, :],
                                 rrB[HD:HD + 1, :], start=True, stop=True)
                nc.vector.tensor_tensor(
                    aT_sb[0:HD, p, t0:t0 + W], aTuA[0:HD, :],
                    rbA[0:HD, :], MULT)
                t64 = pn.tile([HD, W], bf16, tag="t64")
                nc.vector.tensor_tensor(t64[:], aTuB[0:HD, :],
                                        rbB[0:HD, :], MULT)
                if w == NW - 1 and p == NPAIR - 1:
                    t64_last[0] = t64
                else:
                    nc.sync.dma_start(aT_sb[HD:P, p, t0:t0 + W], t64[:])

            return tail

        # ---- phase A: QKV (PE-bound) with window-0 units interleaved ----
        def qkv_chunk(w, what):
            t0 = w * W
            if what in ("q", "k"):
                w_sb, dst = ((wq_sb, qT_sb) if what == "q" else
                             (wk_sb, kT_sb))
                for p in range(NPAIR):
                    ps = pps.tile([P, W], f32, tag="pq")
                    for cc in range(CC):
                        nc.tensor.matmul(
                            ps[:], w_sb[:, cc, p * P:(p + 1) * P],
                            xT_sb[:, cc, t0:t0 + W],
                            start=(cc == 0), stop=(cc == CC - 1))
                    if what == "q":
                        nc.vector.tensor_copy(dst[:, p, t0:t0 + W], ps[:])
                    else:
                        nc.scalar.activation(dst[:, p, t0:t0 + W], ps[:],
                                             COPY)
            else:
                for tcb in range(4):
                    t_c = w * 4 + tcb
                    ps = pps.tile([P, W], f32, tag="pq")
                    for cc in range(CC):
                        nc.tensor.matmul(
                            ps[:], xT_sb[:, cc, t_c * P:(t_c + 1) * P],
                            wv_sb[:, cc, :],
                            start=(cc == 0), stop=(cc == CC - 1))
                    nc.vector.tensor_copy(
                        v_sb[:, t_c, :, 0:HD],
                        ps[:].rearrange("p (h d) -> p h d", h=NH_LOCAL))

        with tc.tile_pool(name=f"pps{r}", bufs=2, space="PSUM") as pps:
            for w in (0, 1):
                for what in ("q", "k", "v"):
                    qkv_chunk(w, what)
            t0_ = emit_unit(0, 0, pps, "pq")
            qkv_chunk(2, "q")
            t0_()
            t1_ = emit_unit(1, 0, pps, "pq")
            qkv_chunk(2, "k")
            t1_()
            t2_ = emit_unit(2, 0, pps, "pq")
            qkv_chunk(2, "v")
            t2_()
            qkv_chunk(3, "q")
            t3_ = emit_unit(3, 0, pps, "pq")
            qkv_chunk(3, "k")
            t3_()
            qkv_chunk(3, "v")

        # ---- phase B: windows 1..3 with pipelined out-proj ----
        with tc.tile_pool(name=f"ppy{r}", bufs=2, space="PSUM") as ppy:
            op_queue = [(fc, 0, False) for fc in range(CC)]

            def emit_op(n):
                for _ in range(n):
                    if not op_queue:
                        return
                    fc, t0, last_w = op_queue.pop(0)
                    py = ppy.tile([P, W], f32, tag="py")
                    for pair in range(NPAIR - 1):
                        nc.tensor.matmul(
                            py[:], wo_sb[:, pair, fc * P:(fc + 1) * P],
                            aT_sb[:, pair, t0:t0 + W],
                            start=(pair == 0), stop=False)
                    if last_w:
                        # pair 3: A half from aT_sb, B half straight from
                        # the staging tile (skips the cross-partition DMA)
                        nc.tensor.matmul(
                            py[:], wo_sb[0:HD, 3, fc * P:(fc + 1) * P],
                            aT_sb[0:HD, 3, t0:t0 + W],
                            start=False, stop=False)
                        nc.tensor.matmul(
                            py[:], woB_sb[:, 3, fc * P:(fc + 1) * P],
                            t64_last[0][:], start=False, stop=True)
                    else:
                        nc.tensor.matmul(
                            py[:], wo_sb[:, NPAIR - 1, fc * P:(fc + 1) * P],
                            aT_sb[:, NPAIR - 1, t0:t0 + W],
                            start=False, stop=True)
                    yst = pys.tile([P, W], bf16, tag="yst")
                    nc.vector.tensor_copy(yst[:], py[:])
                    nc.sync.dma_start(yT_r[:, fc, t0:t0 + W], yst[:])

            prev_tail = None
            for w in range(1, NW):
                for p in range(NPAIR):
                    prev_tail = emit_unit(p, w, ppy, "py", prev_tail)
                    emit_op(2)
                op_queue += [(fc, w * W, w == NW - 1) for fc in range(CC)]
            prev_tail()
            emit_op(len(op_queue))


def _build(repeats=1):
    import concourse.bacc as bacc
    import concourse.mybir as mybir
    import concourse.tile as tile
    from contextlib import ExitStack

    f32 = mybir.dt.float32
    bf16 = mybir.dt.bfloat16

    nc = bacc.Bacc("TRN2", target_bir_lowering=False, debug=False)

    xT = nc.dram_tensor("xT", (D, T), bf16, kind="ExternalInput")
    wqT = nc.dram_tensor("wqT", (D, CL), bf16, kind="ExternalInput")
    wkT = nc.dram_tensor("wkT", (D, CL), bf16, kind="ExternalInput")
    wvT = nc.dram_tensor("wvT", (D, CL), bf16, kind="ExternalInput")
    woT = nc.dram_tensor("woT", (CL, D), bf16, kind="ExternalInput")
    ml = nc.dram_tensor("ml", (P, P), bf16, kind="ExternalInput")
    mr = nc.dram_tensor("mr", (P, P), bf16, kind="ExternalInput")
    yT = nc.dram_tensor("yT", (D, T), bf16, kind="ExternalOutput")

    xT_r = xT.ap().rearrange("(o p) t -> p o t", p=P)
    wqT_r = wqT.ap().rearrange("(o p) f -> p o f", p=P)
    wkT_r = wkT.ap().rearrange("(o p) f -> p o f", p=P)
    wvT_r = wvT.ap().rearrange("(o p) f -> p o f", p=P)
    woT_r = woT.ap().rearrange("(o p) f -> p o f", p=P)
    yT_r = yT.ap().rearrange("(o p) t -> p o t", p=P)

    with tile.TileContext(nc) as tc, ExitStack() as outer:
        persist = outer.enter_context(tc.tile_pool(name="persist", bufs=1))
        xT_sb = persist.tile([P, CC, T], bf16, tag="xT")
        qT_sb = persist.tile([P, NPAIR, T], bf16, tag="qT")
        kT_sb = persist.tile([P, NPAIR, T], bf16, tag="kT")
        aT_sb = persist.tile([P, NPAIR, T], bf16, tag="aT")
        v_sb = persist.tile([P, 16, NH_LOCAL, HD + 1], bf16, tag="v")
        wq_sb = persist.tile([P, CC, CL], bf16, tag="wq")
        wk_sb = persist.tile([P, CC, CL], bf16, tag="wk")
        wv_sb = persist.tile([P, CC, CL], bf16, tag="wv")
        wo_sb = persist.tile([P, NPAIR, D], bf16, tag="wo")
        woB_sb = persist.tile([HD, NPAIR, D], bf16, tag="woB")
        ml_sb = persist.tile([P, P], bf16, tag="ml")
        mr_sb = persist.tile([P, P], bf16, tag="mr")
        ones_sb = persist.tile([P, HD], bf16, tag="ones")
        nc.vector.memset(ones_sb[:], 1.0)

        for r in range(repeats):
            if r == 0:
                # weights + constants load once and stay resident
                nc.sync.dma_start(wq_sb[:], wqT_r)
                nc.sync.dma_start(wk_sb[:], wkT_r)
                nc.scalar.dma_start(wv_sb[:], wvT_r)
                nc.scalar.dma_start(ml_sb[:], ml.ap())
                nc.scalar.dma_start(mr_sb[:], mr.ap())
                nc.scalar.dma_start(wo_sb[:], woT_r)
                nc.scalar.dma_start(woB_sb[:], woT_r[HD:P, :, :])
            for w in range(NW):
                nc.gpsimd.dma_start(xT_sb[:, :, w * W:(w + 1) * W],
                                    xT_r[:, :, w * W:(w + 1) * W])
            nc.vector.memset(v_sb[:, :, :, HD:HD + 1], 1.0)
            _emit_body(nc, tc, mybir, r,
                       (xT_sb, qT_sb, kT_sb, aT_sb, v_sb, wq_sb, wk_sb,
                        wv_sb, wo_sb, woB_sb, ml_sb, mr_sb, ones_sb, yT_r))

    nc.compile()
    return nc


def _host_inputs(x, w_qkv, w_out, core):
    import ml_dtypes

    bf = ml_dtypes.bfloat16
    b, g = core // 2, core % 2
    sl = slice(CL * g, CL * g + CL)
    k_i = np.arange(P)
    # exact boolean causal mask as a rank-127 product:
    #   ml[k,j] = -400*[j == k+1],  mr[k,i] = [i <= k]
    #   => (ml^T @ mr)[j,i] = -400*[j > i]
    # -400 is dead after the 0.125 exp scale (e^-45) yet keeps the ACT exp
    # table input bounded (a -3e3*(j-i) ramp reaches -4.7e4 and breaks the
    # HW exp, which CoreSim's np.exp does not catch).
    ml = np.where(k_i[None, :] == k_i[:, None] + 1, -400.0, 0.0)
    mr = np.where(k_i[None, :] <= k_i[:, None], 1.0, 0.0)
    return {
        "xT": np.ascontiguousarray(x[b].T).astype(bf),
        "wqT": np.ascontiguousarray(w_qkv[0 * D:1 * D][sl].T).astype(bf),
        "wkT": np.ascontiguousarray(w_qkv[1 * D:2 * D][sl].T).astype(bf),
        "wvT": np.ascontiguousarray(w_qkv[2 * D:3 * D][sl].T).astype(bf),
        "woT": np.ascontiguousarray(w_out[:, sl].T).astype(bf),
        "ml": ml.astype(bf),
        "mr": mr.astype(bf),
    }


def kernel(x, w_qkv, w_out):
    from concourse import bass_utils

    if "nc" not in _CACHE:
        _CACHE["nc"] = _build()
    nc = _CACHE["nc"]

    x = np.asarray(x, dtype=np.float32)
    w_qkv = np.asarray(w_qkv, dtype=np.float32)
    w_out = np.asarray(w_out, dtype=np.float32)

    in_maps = [_host_inputs(x, w_qkv, w_out, c) for c in range(8)]
    res = bass_utils.run_bass_kernel_spmd(nc, in_maps, core_ids=list(range(8)))
    outs = res.results

    y = np.empty((B, T, D), dtype=np.float32)
    for b in range(B):
        y[b] = (outs[2 * b]["yT"].astype(np.float32)
                + outs[2 * b + 1]["yT"].astype(np.float32)).T
    return y
